# revision 34
# baseline (speedup 1.0000x reference)
"""Multi-head causal attention (B=4, T=2048, H=16, D=64) on 8 trn2 NeuronCores.

Sharding: core c = (batch b = c//2, head-group hg = c%2 of 8 heads).
Each core computes its batch's QKV projection for its 8 heads, causal
attention, and a partial output projection (contraction over its 512
channels of W_proj). Host sums the two partials per batch and adds bias.

Per-core kernel layout choices:
  - x is passed pre-transposed as xT [C=1024, T=2048].
  - K^T, Q^T stored [hd, t] with head-dim on partitions (64 per head, 2
    heads per 128-partition tile); the per-head score matmul pair (K=64)
    sits on disjoint 64-row PE tiles and dual-issues on HW.
  - V stored [t, h*65+d] with a ones column appended per head -> the AV
    matmul O_T = V_aug^T(stationary) x P_T produces softmax denominators
    in row 64 for free.
  - Scores are computed transposed S_T[k, q]; P_T = exp(S_T) feeds AV
    directly; no max subtraction needed (|scores/8| < ~3).
  - O_T [hd, t] is exactly the lhsT the output projection needs.

Scheduling: the attention inner loop is ACT(exp)-bound while QKV/proj
are PE-bound.  QKV(tb+1)/proj(qb) matmuls are queued as "filler" units
and injected between attention steps via a virtual-clock deficit so the
PE never idles waiting for exps.  Chains are flushed just-in-time one
pair ahead of their consumer so PSUM->SBUF copies (DVE) land early.
Prologue DMAs are split across the two HWDGE queues (SP + ACT).
"""

import os
import sys

import numpy as np

F16_NP = np.dtype(np.float16)

if "/opt/trn_rl_repo" not in sys.path:
    sys.path.insert(0, "/opt/trn_rl_repo")

from collections import deque
from contextlib import ExitStack

import concourse.bass as bass
import concourse.bacc as bacc
import concourse.mybir as mybir
import concourse.tile as tile
from concourse._compat import with_exitstack

P = 128
T = 2048
C = 1024
H_PER_CORE = 8
D = 64
DP = D + 1  # V augmented with a ones column
NC_CORES = 8

TB = 4  # t-blocks of 512
QB = 4  # q-blocks of 512
CI = 8  # contraction tiles of 128 over C for QKV proj

F32 = mybir.dt.float32
F16 = mybir.dt.float16

# virtual-clock cost constants (ns) for filler pacing
PE_CYC = 0.4167
ACT_COL = 0.8333
ACT_OVH = 350.0
STEP_PE_OVH = 35.0
MM_NS = 512 * PE_CYC + 20.0


@with_exitstack
def build_attention_kernel(ctx: ExitStack, tc: tile.TileContext):
    nc = tc.nc

    xT = nc.declare_dram_parameter("xT", [C, T], F16, isOutput=False)
    wk = nc.declare_dram_parameter("wk", [C, 512], F16, isOutput=False)
    wq = nc.declare_dram_parameter("wq", [C, 512], F16, isOutput=False)
    wv = nc.declare_dram_parameter("wv", [C, 512], F16, isOutput=False)
    wp = nc.declare_dram_parameter("wp", [512, C], F16, isOutput=False)
    y = nc.declare_dram_parameter("y", [T, C], F16, isOutput=True)

    xT_t = xT.rearrange("(co ci) t -> ci co t", ci=P)
    wk_t = wk.rearrange("(co ci) m -> ci co m", ci=P)
    wq_t = wq.rearrange("(co ci) m -> ci co m", ci=P)
    wv_t = wv.rearrange("(co ci) m -> ci co m", ci=P)
    wp_t = wp.rearrange("(co ci) n -> ci co n", ci=P)
    y_t = y.rearrange("(tt p) n -> p tt n", p=P)

    # ---- pools ----
    kt_pool = ctx.enter_context(tc.tile_pool(name="ktp", bufs=16))
    qt_pool = ctx.enter_context(tc.tile_pool(name="qtp", bufs=16))
    ot_pool = ctx.enter_context(tc.tile_pool(name="otp", bufs=16))
    v_pool = ctx.enter_context(tc.tile_pool(name="vp", bufs=4))
    const_pool = ctx.enter_context(tc.tile_pool(name="constp", bufs=1))
    w_pool = ctx.enter_context(tc.tile_pool(name="wp_", bufs=1))
    xt_pool = ctx.enter_context(tc.tile_pool(name="xtp", bufs=3))
    pt_pool = ctx.enter_context(tc.tile_pool(name="ptp", bufs=8))
    recip_pool = ctx.enter_context(tc.tile_pool(name="recipp", bufs=6))
    bc_pool = ctx.enter_context(tc.tile_pool(name="bcp", bufs=4))
    y_pool = ctx.enter_context(tc.tile_pool(name="yp", bufs=2))
    # PSUM: 8 banks total: s-pairs 2x2, AV accum 2x1, qkv/proj 2x1
    ps_s_pool = ctx.enter_context(tc.tile_pool(name="ps_s", bufs=2, space="PSUM"))
    ps_o_pool = ctx.enter_context(tc.tile_pool(name="ps_o", bufs=2, space="PSUM"))
    ps_q_pool = ctx.enter_context(tc.tile_pool(name="ps_q", bufs=2, space="PSUM"))

    # KT[pt][tb], QT[pt][qb]: [128, 512]; partitions = 2 heads x 64 dims
    KT = [[kt_pool.tile([P, 512], F16, tag="kt", name=f"KT_{pt}_{tb}") for tb in range(TB)] for pt in range(4)]
    QT = [[qt_pool.tile([P, 512], F16, tag="qt", name=f"QT_{pt}_{qb}") for qb in range(QB)] for pt in range(4)]
    OT = [[ot_pool.tile([P, 512], F16, tag="ot", name=f"OT_{hp}_{qb}") for qb in range(QB)] for hp in range(4)]
    V = [v_pool.tile([P, 4, H_PER_CORE * DP], F16, tag="v", name=f"V_{tb}") for tb in range(TB)]
    masks = const_pool.tile([P, 4, 512], F16, tag="masks", name="masks")
    wk_sb = w_pool.tile([P, CI, 512], F16)
    wq_sb = w_pool.tile([P, CI, 512], F16)
    wv_sb = w_pool.tile([P, CI, 512], F16)
    wp_sb = w_pool.tile([P, 4, C], F16)

    # ---- prologue DMAs: need-ordered slices so pair(0,0) can start after
    # only ~1.6MB has landed (x0 + head-pair-0 slices of wk/wq/wv) ----
    xts = {}
    xt0 = xt_pool.tile([P, CI, 512], F16, tag="xt", name="xt")
    xt1 = xt_pool.tile([P, CI, 512], F16, tag="xt", name="xt")
    nc.sync.dma_start(wk_sb[:, :, 0:128], wk_t[:, :, 0:128])
    nc.scalar.dma_start(xt0[:, :4], xT_t[:, :4, 0:512])
    nc.sync.dma_start(wq_sb[:, :, 0:128], wq_t[:, :, 0:128])
    nc.sync.dma_start(xt0[:, 4:], xT_t[:, 4:, 0:512])
    nc.scalar.dma_start(wv_sb[:, :, 0:256], wv_t[:, :, 0:256])
    nc.sync.dma_start(wk_sb[:, :, 128:256], wk_t[:, :, 128:256])
    nc.sync.dma_start(wq_sb[:, :, 128:256], wq_t[:, :, 128:256])
    nc.sync.dma_start(xt1[:, :4], xT_t[:, :4, 512:1024])
    nc.scalar.dma_start(xt1[:, 4:], xT_t[:, 4:, 512:1024])
    nc.sync.dma_start(wk_sb[:, :, 256:512], wk_t[:, :, 256:512])
    nc.scalar.dma_start(wq_sb[:, :, 256:512], wq_t[:, :, 256:512])
    nc.scalar.dma_start(wv_sb[:, :, 256:512], wv_t[:, :, 256:512])
    nc.gpsimd.dma_start(wp_sb[:], wp_t)
    xts[0] = xt0
    xts[1] = xt1

    # diagonal causal masks: masks[:, j, :][kk, qq] = 1.0 if qq >= kk + j*128
    for j in range(4):
        nc.gpsimd.memset(masks[:, j, :], 1.0)
        nc.gpsimd.affine_select(
            out=masks[:, j, :],
            in_=masks[:, j, :],
            compare_op=mybir.AluOpType.is_ge,
            fill=0.0,
            base=-j * P,
            pattern=[[1, 512]],
            channel_multiplier=-1,
        )
    # ones column of V
    for tb in range(TB):
        ones_col = V[tb].rearrange("p s (h e) -> p s h e", e=DP)[:, :, :, D : D + 1]
        nc.gpsimd.memset(ones_col, 1.0)

    # ================= filler machinery =================
    filler_q = deque()  # entries: (tag, sub, pe_ns, closure)
    clk = {"deficit": 0.0}

    def emit_fillers(gap_ns):
        # 1.2x overfill: surplus PE work lands early (harmless ACT stretch)
        # instead of as PE idle at pair boundaries
        clk["deficit"] += gap_ns * 1.2
        while clk["deficit"] > 0.0 and filler_q:
            _, _, pe_ns, fn = filler_q.popleft()
            fn()
            clk["deficit"] -= pe_ns

    def flush_sub(tag, subs):
        """Emit from the front until no unit with (tag, sub in subs) remains."""
        while any(e[0] == tag and e[1] in subs for e in filler_q):
            _, _, _, fn = filler_q.popleft()
            fn()

    def drain_all():
        while filler_q:
            _, _, _, fn = filler_q.popleft()
            fn()

    # ================= QKV chains =================
    def qkv_chain_units(tb, kind, idx, sub):
        st = {}
        tag = f"qkv{tb}"
        units = []

        def mk(ci):
            def f():
                xt = xts[tb]
                if ci == 0:
                    st["ps"] = ps_q_pool.tile([P, 512], F32, tag="psq", name=f"ps_{tag}")
                ps = st["ps"]
                if kind == "K":
                    nc.tensor.matmul(
                        ps[:], lhsT=wk_sb[:, ci, idx * P : (idx + 1) * P],
                        rhs=xt[:, ci, :], start=(ci == 0), stop=(ci == CI - 1),
                    )
                elif kind == "Q":
                    nc.tensor.matmul(
                        ps[:], lhsT=wq_sb[:, ci, idx * P : (idx + 1) * P],
                        rhs=xt[:, ci, :], start=(ci == 0), stop=(ci == CI - 1),
                    )
                else:  # V
                    nc.tensor.matmul(
                        ps[:], lhsT=xt[:, ci, idx * P : (idx + 1) * P],
                        rhs=wv_sb[:, ci, :], start=(ci == 0), stop=(ci == CI - 1),
                    )
                if ci == CI - 1:
                    if kind == "K":
                        nc.vector.tensor_copy(KT[idx][tb][:], ps[:])
                    elif kind == "Q":
                        nc.vector.tensor_copy(QT[idx][tb][:], ps[:])
                    else:
                        nc.vector.tensor_copy(
                            V[tb][:, idx].rearrange("p (h e) -> p h e", e=DP)[:, :, :D],
                            ps.rearrange("p (h d) -> p h d", d=D),
                        )
            return f

        for ci in range(CI):
            units.append((tag, sub, MM_NS, mk(ci)))
        return units

    def v_half_chain_units(tb, idx, h0):
        """V chain for heads [2*h0, 2*h0+4): N=256 half-width (startup only)."""
        st = {}

        def mk(ci):
            def f():
                xt = xts[tb]
                if ci == 0:
                    st["ps"] = ps_q_pool.tile([P, 512], F32, tag="psq", name="ps_vh")
                ps = st["ps"]
                nc.tensor.matmul(
                    ps[:, :256], lhsT=xt[:, ci, idx * P : (idx + 1) * P],
                    rhs=wv_sb[:, ci, h0 * 256 : (h0 + 1) * 256],
                    start=(ci == 0), stop=(ci == CI - 1),
                )
                if ci == CI - 1:
                    nc.vector.tensor_copy(
                        V[tb][:, idx].rearrange("p (h e) -> p h e", e=DP)[
                            :, 4 * h0 : 4 * h0 + 4, :D
                        ],
                        ps[:, :256].rearrange("p (h d) -> p h d", d=D),
                    )
            return f

        return [(f"qkv{tb}", "v", 256 * PE_CYC + 20.0, mk(ci)) for ci in range(CI)]

    def ldx_unit(tb):
        def f():
            xt = xt_pool.tile([P, CI, 512], F16, tag="xt", name="xt")
            nc.sync.dma_start(xt[:, :4], xT_t[:, :4, tb * 512 : (tb + 1) * 512])
            nc.scalar.dma_start(xt[:, 4:], xT_t[:, 4:, tb * 512 : (tb + 1) * 512])
            xts[tb] = xt
        return (f"qkv{tb}", "v", 0.0, f)

    def qkv_units(tb):
        """V first (long-lead deps), then per-pair (Q,K) chains."""
        units = []
        if tb >= 2:
            units.append(ldx_unit(tb))
        for ts in range(4):
            units += qkv_chain_units(tb, "V", ts, "v")
        for pt in range(4):
            units += qkv_chain_units(tb, "Q", pt, f"p{pt}")
            units += qkv_chain_units(tb, "K", pt, f"p{pt}")
        return units

    # ================= attention =================
    def attention_pair(qb, hp, mid_emit=None):
        # JIT prefetch: flush this pair's chains (safety) plus the next
        # pair's, so the PSUM->SBUF casts land one pair ahead of use.
        subs = {f"p{hp}"}
        if hp == 0:
            subs |= {"v", "p1"}
        elif hp < 3:
            subs.add(f"p{hp + 1}")
        flush_sub(f"qkv{qb}", subs)

        ot_ps = [
            ps_o_pool.tile([DP, 512], F32, tag="ot_ps", name=f"ot_ps_{qb}_{hp}_{i}")
            for i in range(2)
        ]
        nkt = 4 * (qb + 1)
        pts = {}

        def emit_scores_exp(kt):
            tb = kt // 4
            qs = (kt - 4 * qb) * P if kt >= 4 * qb else 0
            nq = 512 - qs
            s_ps = ps_s_pool.tile([P, 2, 512], F32, tag="s_ps", name="s_ps")
            for h2 in range(2):
                nc.tensor.matmul(
                    s_ps[:, h2, qs:],
                    lhsT=KT[hp][tb][
                        h2 * D : (h2 + 1) * D,
                        (kt % 4) * P : (kt % 4 + 1) * P,
                    ],
                    rhs=QT[hp][qb][h2 * D : (h2 + 1) * D, qs:],
                    start=True,
                    stop=True,
                )
            p_t = pt_pool.tile([P, 2, 512], F16, tag="pt", name="p_t")
            nc.scalar.activation(
                p_t[:, :, qs:],
                s_ps[:, :, qs:],
                mybir.ActivationFunctionType.Exp,
                scale=0.125,
            )
            if kt >= 4 * qb:  # diagonal: zero q < k entries (Pool: all-SBUF)
                j = kt - 4 * qb
                mb = masks[:, j : j + 1, qs:].to_broadcast([P, 2, nq])
                nc.gpsimd.tensor_mul(p_t[:, :, qs:], p_t[:, :, qs:], mb)
            pts[kt] = (p_t, qs, nq)
            return nq

        def emit_av(kt):
            tb = kt // 4
            p_t, qs, nq = pts.pop(kt)
            for h2 in range(2):
                h = 2 * hp + h2
                nc.tensor.matmul(
                    ot_ps[h2][:, qs:],
                    lhsT=V[tb][:, kt % 4, h * DP : (h + 1) * DP],
                    rhs=p_t[:, h2, qs:],
                    start=(kt == 0),
                    stop=(kt == nkt - 1),
                )
            return nq

        if mid_emit is not None:
            # split form (startup): all scores/exps first so ACT streams
            # while V is still loading, then the AVs
            for kt in range(nkt):
                nq = emit_scores_exp(kt)
                emit_fillers(2 * nq * ACT_COL + ACT_OVH - nq * PE_CYC - STEP_PE_OVH)
            mid_emit()
            for kt in range(nkt):
                emit_av(kt)
        else:
            # software pipeline: S(kt+1) before AV(kt); fillers pace the gaps
            nq = emit_scores_exp(0)
            emit_fillers(2 * nq * ACT_COL + ACT_OVH - nq * PE_CYC - STEP_PE_OVH)
            for kt in range(1, nkt):
                nq_s = emit_scores_exp(kt)
                nq_a = emit_av(kt - 1)
                emit_fillers(
                    2 * nq_s * ACT_COL + ACT_OVH
                    - (nq_s + 2 * nq_a) * PE_CYC - STEP_PE_OVH
                )
            emit_av(nkt - 1)

        # normalize: divide rows 0..63 by the sums row (64)
        for h2 in range(2):
            recip = recip_pool.tile([1, 512], F32, tag="recip", name="recip")
            nc.vector.tensor_copy(recip[:], ot_ps[h2][D : D + 1, :])
            nc.vector.reciprocal_approx_fast(recip[:], recip[:])
            bc = bc_pool.tile([D, 512], F32, tag="bc", name="bc")
            nc.gpsimd.partition_broadcast(bc[:], recip[:])
            nc.vector.tensor_mul(
                OT[hp][qb][h2 * D : (h2 + 1) * D, :],
                ot_ps[h2][:D, :],
                bc[:],
            )

    # ================= output projection =================
    ysbs = {}

    def proj_units(qb, tts=None, use_ps_s=False):
        tag = f"proj{qb}"
        units = []
        proj_ps = {}

        def mk(tt, nb, ct):
            def f():
                key = tt // 2
                if tt % 2 == 0 and nb == 0 and ct == 0:
                    ysbs[key] = y_pool.tile([P, 2, C], F16, tag="ypair", name="ypair")
                st_key = (tt, nb)
                if ct == 0:
                    if use_ps_s and (tt * 2 + nb) % 2 == 0:
                        proj_ps[st_key] = ps_s_pool.tile(
                            [P, 2, 512], F32, tag="s_ps", name=f"ps_{tag}"
                        )[:, 0, :]
                    else:
                        proj_ps[st_key] = ps_q_pool.tile(
                            [P, 512], F32, tag="psq", name=f"ps_{tag}"
                        )
                ps = proj_ps[st_key]
                nc.tensor.matmul(
                    ps[:],
                    lhsT=OT[ct][qb][:, (tt % 4) * P : (tt % 4 + 1) * P],
                    rhs=wp_sb[:, ct, nb * 512 : (nb + 1) * 512],
                    start=(ct == 0),
                    stop=(ct == 3),
                )
                if ct == 3:
                    proj_ps.pop(st_key)
                    nc.vector.tensor_copy(
                        ysbs[key][:, tt % 2, nb * 512 : (nb + 1) * 512], ps[:]
                    )
                    if tt % 2 == 1 and nb == 1:
                        nc.sync.dma_start(y_t[:, tt - 1 : tt + 1, :], ysbs[key][:])
            return f

        for tt in tts if tts is not None else range(4 * qb, 4 * qb + 4):
            for nb in range(2):
                for ct in range(4):
                    units.append((tag, "p", MM_NS, mk(tt, nb, ct)))
        return units

    # ================= master schedule =================
    # tb=0: K0,V0,Q0 then pair(0,0) ASAP; V1-3 and later K/Q chains slot
    # between the early (cheap) pairs, one pair ahead of their consumers.
    def emit_chains(units):
        for u in units:
            u[3]()

    emit_chains(qkv_chain_units(0, "K", 0, "p0"))
    emit_chains(qkv_chain_units(0, "Q", 0, "p0"))
    for ts in range(4):
        emit_chains(v_half_chain_units(0, ts, 0))
    filler_q.extend(qkv_units(1))
    for hp in range(4):
        if hp == 1:  # second V half feeds pairs hp>=2, one pair ahead
            for ts in range(4):
                emit_chains(v_half_chain_units(0, ts, 1))
        if hp < 3:
            emit_chains(qkv_chain_units(0, "K", hp + 1, f"p{hp+1}"))
            emit_chains(qkv_chain_units(0, "Q", hp + 1, f"p{hp+1}"))
        attention_pair(0, hp)

    for qb in range(1, 4):
        if qb < 3:
            filler_q.extend(qkv_units(qb + 1))
        if qb < 3:
            filler_q.extend(proj_units(qb - 1))
        else:
            # hold back proj(2) tt 10-11 as tail filler: keeps the PE busy
            # (and clocked up) while norm(3,3) runs before proj(3)
            filler_q.extend(proj_units(2, tts=(8, 9)))
        for hp in range(4):
            attention_pair(qb, hp)
    # tail: the ps_s pool is free now; alternate psum pools so the chains
    # pipeline past their casts, and the held-back proj(2) chains fill the
    # PE while norm(3,3) completes
    filler_q.extend(proj_units(2, tts=(10, 11), use_ps_s=True))
    filler_q.extend(proj_units(3, use_ps_s=True))
    drain_all()

    return nc


_CACHED_NC = None


def get_nc():
    global _CACHED_NC
    if _CACHED_NC is None:
        nc = bacc.Bacc()
        with tile.TileContext(nc) as tc:
            build_attention_kernel(tc)
        nc.compile()
        _CACHED_NC = nc
    return _CACHED_NC


def make_in_maps(x, W_att, W_proj):
    x = np.asarray(x, dtype=np.float32)
    W_att = np.asarray(W_att, dtype=np.float32)
    in_maps = []
    for c in range(NC_CORES):
        b, hg = c // 2, c % 2
        s = hg * 512
        in_maps.append(
            {
                "xT": np.ascontiguousarray(x[b].T).astype(F16_NP),
                "wk": np.ascontiguousarray(
                    W_att[:, 0 * C + s : 0 * C + s + 512]
                ).astype(F16_NP),
                "wq": np.ascontiguousarray(
                    W_att[:, 1 * C + s : 1 * C + s + 512]
                ).astype(F16_NP),
                "wv": np.ascontiguousarray(
                    W_att[:, 2 * C + s : 2 * C + s + 512]
                ).astype(F16_NP),
                "wp": np.ascontiguousarray(
                    np.asarray(W_proj, np.float32)[s : s + 512]
                ).astype(F16_NP),
            }
        )
    return in_maps


def combine_outputs(results, b_proj):
    B = NC_CORES // 2
    out = np.empty((B, T, C), dtype=np.float32)
    bias = np.asarray(b_proj, dtype=np.float32)
    for b in range(B):
        out[b] = (
            results[2 * b]["y"].astype(np.float32)
            + results[2 * b + 1]["y"].astype(np.float32)
            + bias
        )
    return out


def kernel(x, W_att, W_proj, b_proj):
    from concourse.bass_utils import run_bass_kernel_spmd

    nc = get_nc()
    in_maps = make_in_maps(x, W_att, W_proj)
    res = run_bass_kernel_spmd(nc, in_maps, list(range(NC_CORES)))
    return combine_outputs(res.results, b_proj)


# revision 37
# speedup vs baseline: 1.8233x; 1.8233x over previous
"""Multi-head causal attention (B=4, T=2048, H=16, D=64) on 8 trn2 NeuronCores.

Sharding: core c = (batch b = c//2, head-group hg = c%2 of 8 heads).
Each core computes its batch's QKV projection for its 8 heads, causal
attention, and a partial output projection (contraction over its 512
channels of W_proj). Host sums the two partials per batch and adds bias.

Per-core kernel layout choices:
  - x is passed pre-transposed as xT [C=1024, T=2048].
  - K^T, Q^T stored [hd, t] with head-dim on partitions (64 per head, 2
    heads per 128-partition tile); the per-head score matmul pair (K=64)
    sits on disjoint 64-row PE tiles and dual-issues on HW.
  - V stored [t, h*65+d] with a ones column appended per head -> the AV
    matmul O_T = V_aug^T(stationary) x P_T produces softmax denominators
    in row 64 for free.
  - Scores are computed transposed S_T[k, q]; P_T = exp(S_T) feeds AV
    directly; no max subtraction needed (|scores/8| < ~3).
  - O_T [hd, t] is exactly the lhsT the output projection needs.

Scheduling: the attention inner loop is ACT(exp)-bound while QKV/proj
are PE-bound.  QKV(tb+1)/proj(qb) matmuls are queued as "filler" units
and injected between attention steps via a virtual-clock deficit so the
PE never idles waiting for exps.  Chains are flushed just-in-time one
pair ahead of their consumer so PSUM->SBUF copies (DVE) land early.
Prologue DMAs are split across the two HWDGE queues (SP + ACT).
"""

import os
import sys

import numpy as np

F16_NP = np.dtype(np.float16)

if "/opt/trn_rl_repo" not in sys.path:
    sys.path.insert(0, "/opt/trn_rl_repo")

from collections import deque
from contextlib import ExitStack

import concourse.bass as bass
import concourse.bacc as bacc
import concourse.mybir as mybir
import concourse.tile as tile
from concourse._compat import with_exitstack

P = 128
T = 2048
C = 1024
H_PER_CORE = 8
D = 64
DP = D + 1  # V augmented with a ones column
NC_CORES = 8

TB = 4  # t-blocks of 512
QB = 4  # q-blocks of 512
CI = 8  # contraction tiles of 128 over C for QKV proj

F32 = mybir.dt.float32
F16 = mybir.dt.float16

# virtual-clock cost constants (ns) for filler pacing
PE_CYC = 0.4167
ACT_COL = 0.8333
ACT_OVH = 350.0
STEP_PE_OVH = 35.0
MM_NS = 512 * PE_CYC + 20.0


@with_exitstack
def build_attention_kernel(ctx: ExitStack, tc: tile.TileContext):
    nc = tc.nc

    xT = nc.declare_dram_parameter("xT", [C, T], F16, isOutput=False)
    wk = nc.declare_dram_parameter("wk", [C, 512], F16, isOutput=False)
    wq = nc.declare_dram_parameter("wq", [C, 512], F16, isOutput=False)
    wv = nc.declare_dram_parameter("wv", [C, 512], F16, isOutput=False)
    wp = nc.declare_dram_parameter("wp", [512, C], F16, isOutput=False)
    y = nc.declare_dram_parameter("y", [T, C], F16, isOutput=True)

    xT_t = xT.rearrange("(co ci) t -> ci co t", ci=P)
    wk_t = wk.rearrange("(co ci) m -> ci co m", ci=P)
    wq_t = wq.rearrange("(co ci) m -> ci co m", ci=P)
    wv_t = wv.rearrange("(co ci) m -> ci co m", ci=P)
    wp_t = wp.rearrange("(co ci) n -> ci co n", ci=P)
    y_t = y.rearrange("(tt p) n -> p tt n", p=P)

    # ---- pools ----
    kt_pool = ctx.enter_context(tc.tile_pool(name="ktp", bufs=16))
    qt_pool = ctx.enter_context(tc.tile_pool(name="qtp", bufs=16))
    ot_pool = ctx.enter_context(tc.tile_pool(name="otp", bufs=16))
    v_pool = ctx.enter_context(tc.tile_pool(name="vp", bufs=4))
    const_pool = ctx.enter_context(tc.tile_pool(name="constp", bufs=1))
    w_pool = ctx.enter_context(tc.tile_pool(name="wp_", bufs=1))
    xt_pool = ctx.enter_context(tc.tile_pool(name="xtp", bufs=3))
    pt_pool = ctx.enter_context(tc.tile_pool(name="ptp", bufs=8))
    recip_pool = ctx.enter_context(tc.tile_pool(name="recipp", bufs=6))
    bc_pool = ctx.enter_context(tc.tile_pool(name="bcp", bufs=4))
    y_pool = ctx.enter_context(tc.tile_pool(name="yp", bufs=2))
    # PSUM: 8 banks total: s-pairs 2x2, AV accum 2x1, qkv/proj 2x1
    ps_s_pool = ctx.enter_context(tc.tile_pool(name="ps_s", bufs=2, space="PSUM"))
    ps_o_pool = ctx.enter_context(tc.tile_pool(name="ps_o", bufs=2, space="PSUM"))
    ps_q_pool = ctx.enter_context(tc.tile_pool(name="ps_q", bufs=2, space="PSUM"))

    # KT[pt][tb], QT[pt][qb]: [128, 512]; partitions = 2 heads x 64 dims
    KT = [[kt_pool.tile([P, 512], F16, tag="kt", name=f"KT_{pt}_{tb}") for tb in range(TB)] for pt in range(4)]
    QT = [[qt_pool.tile([P, 512], F16, tag="qt", name=f"QT_{pt}_{qb}") for qb in range(QB)] for pt in range(4)]
    OT = [[ot_pool.tile([P, 512], F16, tag="ot", name=f"OT_{hp}_{qb}") for qb in range(QB)] for hp in range(4)]
    V = [v_pool.tile([P, 4, H_PER_CORE * DP], F16, tag="v", name=f"V_{tb}") for tb in range(TB)]
    masks = const_pool.tile([P, 4, 512], F16, tag="masks", name="masks")
    wk_sb = w_pool.tile([P, CI, 512], F16)
    wq_sb = w_pool.tile([P, CI, 512], F16)
    wv_sb = w_pool.tile([P, CI, 512], F16)
    wp_sb = w_pool.tile([P, 4, C], F16)

    # ---- prologue DMAs: need-ordered slices so pair(0,0) can start after
    # only ~1.6MB has landed (x0 + head-pair-0 slices of wk/wq/wv) ----
    xts = {}
    xt0 = xt_pool.tile([P, CI, 512], F16, tag="xt", name="xt")
    xt1 = xt_pool.tile([P, CI, 512], F16, tag="xt", name="xt")
    nc.sync.dma_start(wk_sb[:, :, 0:128], wk_t[:, :, 0:128])
    nc.scalar.dma_start(xt0[:, :4], xT_t[:, :4, 0:512])
    nc.sync.dma_start(xt0[:, 4:], xT_t[:, 4:, 0:512])
    nc.sync.dma_start(wq_sb[:, :, 0:128], wq_t[:, :, 0:128])
    nc.scalar.dma_start(wv_sb[:, :, 0:256], wv_t[:, :, 0:256])
    nc.sync.dma_start(wk_sb[:, :, 128:256], wk_t[:, :, 128:256])
    nc.sync.dma_start(wq_sb[:, :, 128:256], wq_t[:, :, 128:256])
    nc.sync.dma_start(xt1[:, :4], xT_t[:, :4, 512:1024])
    nc.scalar.dma_start(xt1[:, 4:], xT_t[:, 4:, 512:1024])
    nc.sync.dma_start(wk_sb[:, :, 256:512], wk_t[:, :, 256:512])
    nc.scalar.dma_start(wq_sb[:, :, 256:512], wq_t[:, :, 256:512])
    nc.scalar.dma_start(wv_sb[:, :, 256:512], wv_t[:, :, 256:512])
    nc.gpsimd.dma_start(wp_sb[:], wp_t)
    xts[0] = xt0
    xts[1] = xt1

    # diagonal causal masks: masks[:, j, :][kk, qq] = 1.0 if qq >= kk + j*128
    for j in range(4):
        nc.gpsimd.memset(masks[:, j, :], 1.0)
        nc.gpsimd.affine_select(
            out=masks[:, j, :],
            in_=masks[:, j, :],
            compare_op=mybir.AluOpType.is_ge,
            fill=0.0,
            base=-j * P,
            pattern=[[1, 512]],
            channel_multiplier=-1,
        )
    # ones column of V
    for tb in range(TB):
        ones_col = V[tb].rearrange("p s (h e) -> p s h e", e=DP)[:, :, :, D : D + 1]
        nc.gpsimd.memset(ones_col, 1.0)

    # ================= filler machinery =================
    filler_q = deque()  # entries: (tag, sub, pe_ns, closure)
    clk = {"deficit": 0.0}

    def emit_fillers(gap_ns):
        # slight overfill: surplus PE work lands early (harmless ACT
        # stretch) instead of as PE idle at pair boundaries
        clk["deficit"] += gap_ns * 1.15
        while clk["deficit"] > 0.0 and filler_q:
            _, _, pe_ns, fn = filler_q.popleft()
            fn()
            clk["deficit"] -= pe_ns

    def flush_sub(tag, subs):
        """Emit from the front until no unit with (tag, sub in subs) remains."""
        while any(e[0] == tag and e[1] in subs for e in filler_q):
            _, _, _, fn = filler_q.popleft()
            fn()

    def drain_all():
        while filler_q:
            _, _, _, fn = filler_q.popleft()
            fn()

    # ================= QKV chains =================
    def qkv_chain_units(tb, kind, idx, sub):
        st = {}
        tag = f"qkv{tb}"
        units = []

        def mk(ci):
            def f():
                xt = xts[tb]
                if ci == 0:
                    st["ps"] = ps_q_pool.tile([P, 512], F32, tag="psq", name=f"ps_{tag}")
                ps = st["ps"]
                if kind == "K":
                    nc.tensor.matmul(
                        ps[:], lhsT=wk_sb[:, ci, idx * P : (idx + 1) * P],
                        rhs=xt[:, ci, :], start=(ci == 0), stop=(ci == CI - 1),
                    )
                elif kind == "Q":
                    nc.tensor.matmul(
                        ps[:], lhsT=wq_sb[:, ci, idx * P : (idx + 1) * P],
                        rhs=xt[:, ci, :], start=(ci == 0), stop=(ci == CI - 1),
                    )
                else:  # V
                    nc.tensor.matmul(
                        ps[:], lhsT=xt[:, ci, idx * P : (idx + 1) * P],
                        rhs=wv_sb[:, ci, :], start=(ci == 0), stop=(ci == CI - 1),
                    )
                if ci == CI - 1:
                    if kind == "K":
                        nc.vector.tensor_copy(KT[idx][tb][:], ps[:])
                    elif kind == "Q":
                        nc.vector.tensor_copy(QT[idx][tb][:], ps[:])
                    else:
                        nc.vector.tensor_copy(
                            V[tb][:, idx].rearrange("p (h e) -> p h e", e=DP)[:, :, :D],
                            ps.rearrange("p (h d) -> p h d", d=D),
                        )
            return f

        for ci in range(CI):
            units.append((tag, sub, MM_NS, mk(ci)))
        return units

    def v_half_chain_units(tb, idx, h0):
        """V chain for heads [2*h0, 2*h0+4): N=256 half-width (startup only)."""
        st = {}

        def mk(ci):
            def f():
                xt = xts[tb]
                if ci == 0:
                    st["ps"] = ps_q_pool.tile([P, 512], F32, tag="psq", name="ps_vh")
                ps = st["ps"]
                nc.tensor.matmul(
                    ps[:, :256], lhsT=xt[:, ci, idx * P : (idx + 1) * P],
                    rhs=wv_sb[:, ci, h0 * 256 : (h0 + 1) * 256],
                    start=(ci == 0), stop=(ci == CI - 1),
                )
                if ci == CI - 1:
                    nc.vector.tensor_copy(
                        V[tb][:, idx].rearrange("p (h e) -> p h e", e=DP)[
                            :, 4 * h0 : 4 * h0 + 4, :D
                        ],
                        ps[:, :256].rearrange("p (h d) -> p h d", d=D),
                    )
            return f

        return [(f"qkv{tb}", "v", 256 * PE_CYC + 20.0, mk(ci)) for ci in range(CI)]

    def ldx_unit(tb):
        def f():
            xt = xt_pool.tile([P, CI, 512], F16, tag="xt", name="xt")
            nc.sync.dma_start(xt[:, :4], xT_t[:, :4, tb * 512 : (tb + 1) * 512])
            nc.scalar.dma_start(xt[:, 4:], xT_t[:, 4:, tb * 512 : (tb + 1) * 512])
            xts[tb] = xt
        return (f"qkv{tb}", "v", 0.0, f)

    def qkv_units(tb):
        """V first (long-lead deps), then per-pair (Q,K) chains."""
        units = []
        if tb >= 2:
            units.append(ldx_unit(tb))
        for ts in range(4):
            units += qkv_chain_units(tb, "V", ts, "v")
        for pt in range(4):
            units += qkv_chain_units(tb, "Q", pt, f"p{pt}")
            units += qkv_chain_units(tb, "K", pt, f"p{pt}")
        return units

    # ================= attention =================
    def attention_pair(qb, hp, mid_emit=None):
        # JIT prefetch: flush this pair's chains (safety) plus the next
        # pair's, so the PSUM->SBUF casts land one pair ahead of use.
        subs = {f"p{hp}"}
        if hp == 0:
            subs |= {"v", "p1"}
        elif hp < 3:
            subs.add(f"p{hp + 1}")
        flush_sub(f"qkv{qb}", subs)

        ot_ps = [
            ps_o_pool.tile([DP, 512], F32, tag="ot_ps", name=f"ot_ps_{qb}_{hp}_{i}")
            for i in range(2)
        ]
        nkt = 4 * (qb + 1)
        pts = {}

        def emit_scores_exp(kt):
            tb = kt // 4
            qs = (kt - 4 * qb) * P if kt >= 4 * qb else 0
            nq = 512 - qs
            s_ps = ps_s_pool.tile([P, 2, 512], F32, tag="s_ps", name="s_ps")
            for h2 in range(2):
                nc.tensor.matmul(
                    s_ps[:, h2, qs:],
                    lhsT=KT[hp][tb][
                        h2 * D : (h2 + 1) * D,
                        (kt % 4) * P : (kt % 4 + 1) * P,
                    ],
                    rhs=QT[hp][qb][h2 * D : (h2 + 1) * D, qs:],
                    start=True,
                    stop=True,
                )
            p_t = pt_pool.tile([P, 2, 512], F16, tag="pt", name="p_t")
            nc.scalar.activation(
                p_t[:, :, qs:],
                s_ps[:, :, qs:],
                mybir.ActivationFunctionType.Exp,
                scale=0.125,
            )
            if kt >= 4 * qb:  # diagonal: zero q < k entries
                j = kt - 4 * qb
                mb = masks[:, j : j + 1, qs:].to_broadcast([P, 2, nq])
                nc.vector.tensor_mul(p_t[:, :, qs:], p_t[:, :, qs:], mb)
            pts[kt] = (p_t, qs, nq)
            return nq

        def emit_av(kt):
            tb = kt // 4
            p_t, qs, nq = pts.pop(kt)
            for h2 in range(2):
                h = 2 * hp + h2
                nc.tensor.matmul(
                    ot_ps[h2][:, qs:],
                    lhsT=V[tb][:, kt % 4, h * DP : (h + 1) * DP],
                    rhs=p_t[:, h2, qs:],
                    start=(kt == 0),
                    stop=(kt == nkt - 1),
                )
            return nq

        if mid_emit is not None:
            # split form (startup): all scores/exps first so ACT streams
            # while V is still loading, then the AVs
            for kt in range(nkt):
                nq = emit_scores_exp(kt)
                emit_fillers(2 * nq * ACT_COL + ACT_OVH - nq * PE_CYC - STEP_PE_OVH)
            mid_emit()
            for kt in range(nkt):
                emit_av(kt)
        else:
            # software pipeline: S(kt+1) before AV(kt); fillers pace the gaps
            nq = emit_scores_exp(0)
            emit_fillers(2 * nq * ACT_COL + ACT_OVH - nq * PE_CYC - STEP_PE_OVH)
            for kt in range(1, nkt):
                nq_s = emit_scores_exp(kt)
                nq_a = emit_av(kt - 1)
                emit_fillers(
                    2 * nq_s * ACT_COL + ACT_OVH
                    - (nq_s + 2 * nq_a) * PE_CYC - STEP_PE_OVH
                )
            emit_av(nkt - 1)

        # normalize: divide rows 0..63 by the sums row (64)
        for h2 in range(2):
            recip = recip_pool.tile([1, 512], F32, tag="recip", name="recip")
            nc.vector.tensor_copy(recip[:], ot_ps[h2][D : D + 1, :])
            nc.vector.reciprocal_approx_fast(recip[:], recip[:])
            bc = bc_pool.tile([D, 512], F32, tag="bc", name="bc")
            nc.gpsimd.partition_broadcast(bc[:], recip[:])
            nc.vector.tensor_mul(
                OT[hp][qb][h2 * D : (h2 + 1) * D, :],
                ot_ps[h2][:D, :],
                bc[:],
            )

    # ================= output projection =================
    ysbs = {}

    def proj_units(qb, tts=None, use_ps_s=False):
        tag = f"proj{qb}"
        units = []
        proj_ps = {}

        def mk(tt, nb, ct):
            def f():
                key = tt // 2
                if tt % 2 == 0 and nb == 0 and ct == 0:
                    ysbs[key] = y_pool.tile([P, 2, C], F16, tag="ypair", name="ypair")
                st_key = (tt, nb)
                if ct == 0:
                    if use_ps_s and (tt * 2 + nb) % 2 == 0:
                        proj_ps[st_key] = ps_s_pool.tile(
                            [P, 2, 512], F32, tag="s_ps", name=f"ps_{tag}"
                        )[:, 0, :]
                    else:
                        proj_ps[st_key] = ps_q_pool.tile(
                            [P, 512], F32, tag="psq", name=f"ps_{tag}"
                        )
                ps = proj_ps[st_key]
                nc.tensor.matmul(
                    ps[:],
                    lhsT=OT[ct][qb][:, (tt % 4) * P : (tt % 4 + 1) * P],
                    rhs=wp_sb[:, ct, nb * 512 : (nb + 1) * 512],
                    start=(ct == 0),
                    stop=(ct == 3),
                )
                if ct == 3:
                    proj_ps.pop(st_key)
                    nc.vector.tensor_copy(
                        ysbs[key][:, tt % 2, nb * 512 : (nb + 1) * 512], ps[:]
                    )
                    if tt % 2 == 1 and nb == 1:
                        nc.sync.dma_start(y_t[:, tt - 1 : tt + 1, :], ysbs[key][:])
            return f

        for tt in tts if tts is not None else range(4 * qb, 4 * qb + 4):
            for nb in range(2):
                for ct in range(4):
                    units.append((tag, "p", MM_NS, mk(tt, nb, ct)))
        return units

    # ================= master schedule =================
    # tb=0: K0,V0,Q0 then pair(0,0) ASAP; V1-3 and later K/Q chains slot
    # between the early (cheap) pairs, one pair ahead of their consumers.
    def emit_chains(units):
        for u in units:
            u[3]()

    emit_chains(qkv_chain_units(0, "K", 0, "p0"))
    emit_chains(qkv_chain_units(0, "Q", 0, "p0"))
    for ts in range(4):
        emit_chains(v_half_chain_units(0, ts, 0))
    filler_q.extend(qkv_units(1))
    for hp in range(4):
        if hp == 1:  # second V half feeds pairs hp>=2, one pair ahead
            for ts in range(4):
                emit_chains(v_half_chain_units(0, ts, 1))
        if hp < 3:
            emit_chains(qkv_chain_units(0, "K", hp + 1, f"p{hp+1}"))
            emit_chains(qkv_chain_units(0, "Q", hp + 1, f"p{hp+1}"))
        attention_pair(0, hp)

    for qb in range(1, 4):
        if qb < 3:
            filler_q.extend(qkv_units(qb + 1))
        if qb < 3:
            filler_q.extend(proj_units(qb - 1))
        else:
            # hold back proj(2) tt 10-11 as tail filler: keeps the PE busy
            # (and clocked up) while norm(3,3) runs before proj(3)
            filler_q.extend(proj_units(2, tts=(8, 9)))
        for hp in range(4):
            attention_pair(qb, hp)
    # tail: the ps_s pool is free now; alternate psum pools so the chains
    # pipeline past their casts, and the held-back proj(2) chains fill the
    # PE while norm(3,3) completes
    filler_q.extend(proj_units(2, tts=(10, 11), use_ps_s=True))
    filler_q.extend(proj_units(3, use_ps_s=True))
    drain_all()

    return nc


_CACHED_NC = None


def get_nc():
    global _CACHED_NC
    if _CACHED_NC is None:
        nc = bacc.Bacc()
        with tile.TileContext(nc) as tc:
            build_attention_kernel(tc)
        nc.compile()
        _CACHED_NC = nc
    return _CACHED_NC


def make_in_maps(x, W_att, W_proj):
    x = np.asarray(x, dtype=np.float32)
    W_att = np.asarray(W_att, dtype=np.float32)
    in_maps = []
    for c in range(NC_CORES):
        b, hg = c // 2, c % 2
        s = hg * 512
        in_maps.append(
            {
                "xT": np.ascontiguousarray(x[b].T).astype(F16_NP),
                "wk": np.ascontiguousarray(
                    W_att[:, 0 * C + s : 0 * C + s + 512]
                ).astype(F16_NP),
                "wq": np.ascontiguousarray(
                    W_att[:, 1 * C + s : 1 * C + s + 512]
                ).astype(F16_NP),
                "wv": np.ascontiguousarray(
                    W_att[:, 2 * C + s : 2 * C + s + 512]
                ).astype(F16_NP),
                "wp": np.ascontiguousarray(
                    np.asarray(W_proj, np.float32)[s : s + 512]
                ).astype(F16_NP),
            }
        )
    return in_maps


def combine_outputs(results, b_proj):
    B = NC_CORES // 2
    out = np.empty((B, T, C), dtype=np.float32)
    bias = np.asarray(b_proj, dtype=np.float32)
    for b in range(B):
        out[b] = (
            results[2 * b]["y"].astype(np.float32)
            + results[2 * b + 1]["y"].astype(np.float32)
            + bias
        )
    return out


def kernel(x, W_att, W_proj, b_proj):
    from concourse.bass_utils import run_bass_kernel_spmd

    nc = get_nc()
    in_maps = make_in_maps(x, W_att, W_proj)
    res = run_bass_kernel_spmd(nc, in_maps, list(range(NC_CORES)))
    return combine_outputs(res.results, b_proj)
